# revision 30
# baseline (speedup 1.0000x reference)
"""Trainium2 Bass kernel for nn_Conv2dGeneral (capsule-style 4x4-pose conv).

Math (from the reference):
  out[b,o,X,Y,u,w] = sum_{cin,kx,ky,v} Wm[(cin,kx,ky),o,u,v] * x[b,cin,2X+kx,2Y+ky,4v+w] + bias[o]

Mapped to the PE array as a single 1152-deep contraction:
  K = (cin, v)  x  9 accumulation chunks over (kx, ky)   [9 x 128]
  M = (o, u)                                              [128 PSUM partitions]
  N = (X, Y, w)                                           [676 per batch image]

Data-parallel across 8 NeuronCores on the batch dim (8 images per core).

Pipelining: weights + all 8 images are packed into ONE fp16 DRAM buffer per
core, streamed via column-range DMA chunks (img0 in row-quarters so compute
starts as early as possible, imgs 1-7 whole). Row/col 27 of x are dead
(stride-2 3-tap windows over 28 touch only 0..26) and dropped host-side.
The PE warms its HAM clock gate on a memset scratch region while chunk 0 is
in flight. Outputs are evicted per-group in fp16 (bias re-added host-side)
and shipped per-image by the SP engine AFTER all input triggers: HWDGE ring
FIFO then guarantees output traffic never delays input streaming.
"""

import numpy as np

B, CIN, COUT = 64, 32, 32
KK, STRIDE = 3, 2
WIN, HH = 28, 16
H = 4
WU = 27                          # used rows/cols (row 27 never read)
WOUT = (WIN - KK) // STRIDE + 1  # 13
NCORES = 8
BPC = B // NCORES                # batches per core
RCW = WU * WU * H                # 2916 free elements per (cin,v) partition
RL = WU * H                      # 108 elems per row
NOUT = WOUT * WOUT * H           # 676 outputs per (o,u) partition per image
XSPLIT = ((0, 4), (4, 4), (8, 5))  # X groups per image (208/208/260 cols)
GPB = len(XSPLIT)
NG = GPB * BPC                   # 24 accumulation groups
HALF = 2 * WOUT * H * 4          # 416: output cols of the first two X groups
WARMUP = 30                      # PE warm-up matmuls; must BRIDGE until chunk 0
                                 # lands (an idle gap before the first real
                                 # matmul resets the HAM busy-window and the
                                 # whole first image then runs at 1.2 GHz)

OFF_X = 9 * 128                  # [wt(1152) | img0..7(2916 each)]
NELEM = OFF_X + BPC * RCW

# DMA chunks (elem ranges) on the SP ring: img0 in row-thirds matching
# XSPLIT row needs (X rows 0-3 read x-rows 0-8, X 4-7 read 8-16, X 8-12
# read 16-26), img1 in a 9/18-row split (kills the early-pipeline bubble:
# its first rows land before img0's compute drains), imgs 2-7 whole.
# The weights stream CONCURRENTLY on the ACT ring. Each chunk has its OWN
# completion semaphore: a single cumulative sem is racy because the 16
# per-SDMA-engine increments of back-to-back DMAs interleave, so sem>=16
# does not imply the FIRST dma finished. Per-engine FIFO order does make
# "chunk c done" imply all earlier chunks done on that ring.
_X1 = OFF_X + RCW
_CHUNKS = [
    (0, OFF_X),                          # weights: first on the SP ring
    (OFF_X + 9 * RL, OFF_X + 17 * RL),   # img0 rows 9-16 (rows 0-8 ride ACT)
    (OFF_X + 17 * RL, OFF_X + RCW),
    (_X1, _X1 + 9 * RL),
    (_X1 + 9 * RL, _X1 + RCW),
]
for _b in range(2, BPC):
    _o = OFF_X + _b * RCW
    _CHUNKS.append((_o, _o + RCW))
NCHUNK = len(_CHUNKS)


def _chunk_needed(b, t):
    # index of the last SP-ring DMA chunk group (b, t) requires
    # (group (0,0) needs none beyond chunk 0: its rows ride the ACT ring)
    if b == 0:
        return t
    if b == 1:
        return 3 if t == 0 else 4
    return b + 3

_cache = {}


def _build_bass():
    """Raw-bass build (no Tile): this toolchain's walrus codegen allows only
    ONE sync-wait per instruction, so all cross-engine sync is explicit
    single-sem waits; ordering beyond that rides on hardware transitivity.

    Engines: SP triggers 11 input DMA chunks then 8 per-image output DMAs
    (same HWDGE ring: FIFO keeps outputs behind all inputs), PE runs 25
    accumulation groups of 9 matmuls (one per kernel tap), ACT evicts
    PSUM->SBUF in fp16.
    """
    from contextlib import ExitStack

    import concourse.bass as bass
    import concourse.mybir as mybir

    f32 = mybir.dt.float32
    f16 = mybir.dt.float16

    nc = bass.Bass()
    xin = nc.declare_dram_parameter("xin", [128, NELEM], f16, isOutput=False)
    o_d = nc.declare_dram_parameter("out", [BPC, 128, NOUT], f16, isOutput=True)

    with (
        ExitStack() as stack,
        nc.sbuf_tensor([128, NELEM], f16) as allt,
        nc.sbuf_tensor([128, BPC, NOUT], f16) as ot,
        nc.psum_tensor([128, 8, 512], f32) as ps,
        nc.semaphore("pe_sem") as pe_sem,
        nc.semaphore("act_sem") as act_sem,
        nc.semaphore("out_sem") as out_sem,
        nc.semaphore("warm_sem") as warm_sem,
        nc.semaphore("wt_sem") as wt_sem,
        nc.Block(no_gpsimd_drain=True) as block,
    ):
        c_sems = [
            stack.enter_context(nc.semaphore(f"c_sem{i}")) for i in range(NCHUNK)
        ]
        wtr = allt[:, 0 : 9 * 128].rearrange("p (k m) -> p k m", k=9)

        @block.sync
        def _(sync):
            for c, (a0, a1) in enumerate(_CHUNKS):
                sync.dma_start(allt[:, a0:a1], xin[:, a0:a1]).then_inc(c_sems[c], 16)
            # Ship the LAST image in two halves from here: SP is idle, so
            # the final (small) transfer starts the moment its eviction
            # lands instead of queueing behind ACT's trigger issue.
            sync.wait_ge(act_sem, NG - 1)
            sync.dma_start(
                o_d[BPC - 1][:, :HALF], ot[:, BPC - 1, :HALF]
            ).then_inc(out_sem, 16)
            sync.wait_ge(act_sem, NG)
            sync.dma_start(
                o_d[BPC - 1][:, HALF:], ot[:, BPC - 1, HALF:]
            ).then_inc(out_sem, 16)
            sync.wait_ge(out_sem, 16 * (BPC + 1))

        @block.vector
        def _(vector):
            # Zero the warm-up operand region: reading never-written SBUF
            # trips the sim (and is unhealthy on hardware).
            vector.memset(ot[:, 0, :128], 0).then_inc(warm_sem, 1)

        @block.tensor
        def _(tensor):
            # Warm the PE HAM clock gate (cold = 1.2 GHz) on zeros while
            # chunk 0 (weights + img0 rows 0-8) streams in.
            tensor.wait_ge(warm_sem, 1)
            for i in range(WARMUP):
                tensor.matmul(
                    ps[:, 7, :128], ot[:, 0, :128], ot[:, 0, :128],
                    start=True, stop=True,
                )
            tensor.wait_ge(wt_sem, 16)  # img0 rows 0-8 (ACT ring)
            prev_need = -1
            for j in range(NG):
                b, t = divmod(j, GPB)
                need = _chunk_needed(b, t)
                if need > prev_need:
                    tensor.wait_ge(c_sems[need], 16)
                    prev_need = need
                if t == 0 and 3 * b - 5 >= 1:
                    # banks for this image's 3 groups are free once ACT
                    # drained groups 8 back (one coarse wait per image)
                    tensor.wait_ge(act_sem, 3 * b - 5)
                X0, nX = XSPLIT[t]
                gr = allt[:, OFF_X + b * RCW : OFF_X + (b + 1) * RCW].rearrange(
                    "p (r c w) -> p r c w", r=WU, c=WU
                )
                for kk in range(9):
                    kx, ky = divmod(kk, 3)
                    rhs = gr[
                        :,
                        2 * X0 + kx : 2 * X0 + kx + 2 * nX - 1 : 2,
                        ky : ky + 2 * WOUT - 1 : 2,
                        :,
                    ]
                    mm = tensor.matmul(
                        ps[:, j % 8, : nX * WOUT * H],
                        wtr[:, kk, :],
                        rhs,
                        start=(kk == 0),
                        stop=(kk == 8),
                    )
                mm.then_inc(pe_sem, 1)

        @block.scalar
        def _(scalar):
            # img0 rows 0-8 stream on THIS ring, concurrent with wt on SP
            scalar.dma_start(
                allt[:, OFF_X : OFF_X + 9 * RL], xin[:, OFF_X : OFF_X + 9 * RL]
            ).then_inc(wt_sem, 16)
            for j in range(NG):
                b, t = divmod(j, GPB)
                X0, nX = XSPLIT[t]
                off = X0 * WOUT * H
                scalar.wait_ge(pe_sem, j + 1)
                scalar.activation(
                    ot[:, b, off : off + nX * WOUT * H],
                    ps[:, j % 8, : nX * WOUT * H],
                    mybir.ActivationFunctionType.Copy,
                ).then_inc(act_sem, 1)
                if t == GPB - 1 and b < BPC - 1:
                    # image complete; ship it (the wait orders the async DMA
                    # read after this engine's writes for the race checker)
                    scalar.wait_ge(act_sem, j + 1)
                    scalar.dma_start(o_d[b], ot[:, b, :]).then_inc(out_sem, 16)

    return nc


def _prep_inputs(x, W, bias):
    """Build per-core [128, NELEM] fp16 input buffers.

    fp16: PE runs fp32 matmuls as LOW_HIGH double passes; fp16 is single-pass
    with fast-weight-load, and halves the dominant HBM traffic. Max rel err
    ~5e-4 at this contraction depth (fp32 PSUM accumulation).
    """
    x = np.asarray(x, dtype=np.float32)
    # xp[b, cin*4+v, (r*27+c)*4+w] = x[b,cin,r,c,4v+w], r/c < 27
    xp = np.ascontiguousarray(
        x.reshape(B, CIN, WIN, WIN, H, H)[:, :, :WU, :WU]
        .transpose(0, 1, 4, 2, 3, 5)
    ).reshape(B, CIN * H, RCW).astype(np.float16)
    # W: (1, 288, 32, 1, 1, 4, 4); p = cin*9 + kx*3 + ky
    # wt_sb[cin*4+v, kk*128 + o*4+u] = Wm[cin*9+kk, o, u, v]
    Wm = np.asarray(W, dtype=np.float32).reshape(CIN, KK * KK, COUT, H, H)
    wt_sb = np.ascontiguousarray(
        Wm.transpose(0, 4, 1, 2, 3)  # cin, v, kk, o, u
    ).reshape(128, 9 * 128).astype(np.float16)
    bufs = []
    for core in range(NCORES):
        shard = xp[core * BPC : (core + 1) * BPC]  # (BPC, 128, RCW)
        bufs.append(
            np.ascontiguousarray(
                np.concatenate(
                    [wt_sb, shard.transpose(1, 0, 2).reshape(128, BPC * RCW)],
                    axis=1,
                )
            )
        )
    return bufs


def _make_in_maps(x, W, bias):
    return [{"xin": buf} for buf in _prep_inputs(x, W, bias)]


def _unprep_output(full, bias):
    # full: (B, 128, NOUT) fp16 with partition o*4+u, free (X, Y, w).
    # Bias (a per-channel constant) is added host-side to keep the device
    # eviction a plain fp16 Copy.
    out = (
        full.astype(np.float32)
        .reshape(B, COUT, H, WOUT, WOUT, H)
        .transpose(0, 1, 3, 4, 2, 5)
        .reshape(B, COUT, WOUT, WOUT, HH)
    )
    out += np.asarray(bias, dtype=np.float32).reshape(1, COUT, 1, 1, 1)
    return np.ascontiguousarray(out)


def run_device(in_maps, trace=False, tmpdir=None):
    from concourse.bass_utils import run_bass_kernel_spmd

    if "nc" not in _cache:
        _cache["nc"] = _build_bass()
    return run_bass_kernel_spmd(
        _cache["nc"], in_maps, list(range(NCORES)), trace=trace, tmpdir=tmpdir
    )


def kernel(x, W, bias):
    in_maps = _make_in_maps(x, W, bias)
    res = run_device(in_maps, trace=False)
    full = np.concatenate(
        [np.asarray(res.results[i]["out"]) for i in range(NCORES)], axis=0
    )
    return _unprep_output(full, bias)


# revision 35
# speedup vs baseline: 1.0812x; 1.0812x over previous
"""Trainium2 Bass kernel for nn_Conv2dGeneral (capsule-style 4x4-pose conv).

Math (from the reference):
  out[b,o,X,Y,u,w] = sum_{cin,kx,ky,v} Wm[(cin,kx,ky),o,u,v] * x[b,cin,2X+kx,2Y+ky,4v+w] + bias[o]

Mapped to the PE array as a single 1152-deep contraction:
  K = (cin, v)  x  9 accumulation chunks over (kx, ky)   [9 x 128]
  M = (o, u)                                              [128 PSUM partitions]
  N = (X, Y, w)                                           [676 per batch image]

Data-parallel across 8 NeuronCores on the batch dim (8 images per core).

Pipelining: weights + all 8 images are packed into ONE fp16 DRAM buffer per
core, streamed via column-range DMA chunks (img0 in row-quarters so compute
starts as early as possible, imgs 1-7 whole). Row/col 27 of x are dead
(stride-2 3-tap windows over 28 touch only 0..26) and dropped host-side.
The PE warms its HAM clock gate on a memset scratch region while chunk 0 is
in flight. Outputs are evicted per-group in fp16 (bias re-added host-side)
and shipped per-image by the SP engine AFTER all input triggers: HWDGE ring
FIFO then guarantees output traffic never delays input streaming.
"""

import numpy as np

B, CIN, COUT = 64, 32, 32
KK, STRIDE = 3, 2
WIN, HH = 28, 16
H = 4
WU = 27                          # used rows/cols (row 27 never read)
WOUT = (WIN - KK) // STRIDE + 1  # 13
NCORES = 8
BPC = B // NCORES                # batches per core
RCW = WU * WU * H                # 2916 free elements per (cin,v) partition
RL = WU * H                      # 108 elems per row
NOUT = WOUT * WOUT * H           # 676 outputs per (o,u) partition per image
XSPLIT = ((0, 4), (4, 4), (8, 5))  # X groups per image (208/208/260 cols)
GPB = len(XSPLIT)
NG = GPB * BPC                   # 24 accumulation groups
HALF = 2 * WOUT * H * 4          # 416: output cols of the first two X groups
WARMUP = 42                      # PE warm-up matmuls; must BRIDGE until chunk 0
                                 # lands (an idle gap before the first real
                                 # matmul resets the HAM busy-window and the
                                 # whole first image then runs at 1.2 GHz)

OFF_X = 9 * 128                  # [wt(1152) | img0..7(2916 each)]
NELEM = OFF_X + BPC * RCW

# DMA chunks (elem ranges) on the SP ring: img0 in row-thirds matching
# XSPLIT row needs (X rows 0-3 read x-rows 0-8, X 4-7 read 8-16, X 8-12
# read 16-26), img1 in a 9/18-row split (kills the early-pipeline bubble:
# its first rows land before img0's compute drains), imgs 2-7 whole.
# The weights stream CONCURRENTLY on the ACT ring. Each chunk has its OWN
# completion semaphore: a single cumulative sem is racy because the 16
# per-SDMA-engine increments of back-to-back DMAs interleave, so sem>=16
# does not imply the FIRST dma finished. Per-engine FIFO order does make
# "chunk c done" imply all earlier chunks done on that ring.
_X1 = OFF_X + RCW
_CHUNKS = [
    (0, OFF_X + 9 * RL),                 # weights + img0 rows 0-8
    (OFF_X + 9 * RL, OFF_X + 17 * RL),
    (OFF_X + 17 * RL, OFF_X + RCW),
    (_X1, _X1 + 9 * RL),
    (_X1 + 9 * RL, _X1 + RCW),
]
for _b in range(2, BPC):
    _o = OFF_X + _b * RCW
    _CHUNKS.append((_o, _o + RCW))
NCHUNK = len(_CHUNKS)


def _chunk_needed(b, t):
    # index of the last DMA chunk group (b, t) requires
    if b == 0:
        return t
    if b == 1:
        return 3 if t == 0 else 4
    return b + 3

_cache = {}


def _build_bass():
    """Raw-bass build (no Tile): this toolchain's walrus codegen allows only
    ONE sync-wait per instruction, so all cross-engine sync is explicit
    single-sem waits; ordering beyond that rides on hardware transitivity.

    Engines: SP triggers 11 input DMA chunks then 8 per-image output DMAs
    (same HWDGE ring: FIFO keeps outputs behind all inputs), PE runs 25
    accumulation groups of 9 matmuls (one per kernel tap), ACT evicts
    PSUM->SBUF in fp16.
    """
    from contextlib import ExitStack

    import concourse.bass as bass
    import concourse.mybir as mybir

    f32 = mybir.dt.float32
    f16 = mybir.dt.float16

    nc = bass.Bass()
    xin = nc.declare_dram_parameter("xin", [128, NELEM], f16, isOutput=False)
    o_d = nc.declare_dram_parameter("out", [BPC, 128, NOUT], f16, isOutput=True)

    with (
        ExitStack() as stack,
        nc.sbuf_tensor([128, NELEM], f16) as allt,
        nc.sbuf_tensor([128, BPC, NOUT], f16) as ot,
        nc.psum_tensor([128, 8, 512], f32) as ps,
        nc.semaphore("pe_sem") as pe_sem,
        nc.semaphore("act_sem") as act_sem,
        nc.semaphore("out_sem") as out_sem,
        nc.semaphore("warm_sem") as warm_sem,
        nc.Block(no_gpsimd_drain=True) as block,
    ):
        c_sems = [
            stack.enter_context(nc.semaphore(f"c_sem{i}")) for i in range(NCHUNK)
        ]
        wtr = allt[:, 0 : 9 * 128].rearrange("p (k m) -> p k m", k=9)

        @block.sync
        def _(sync):
            for c, (a0, a1) in enumerate(_CHUNKS):
                sync.dma_start(allt[:, a0:a1], xin[:, a0:a1]).then_inc(c_sems[c], 16)
            # Ship the LAST image in two halves from here: SP is idle, so
            # the final (small) transfer starts the moment its eviction
            # lands instead of queueing behind ACT's trigger issue.
            sync.wait_ge(act_sem, NG - 1)
            sync.dma_start(
                o_d[BPC - 1][:, :HALF], ot[:, BPC - 1, :HALF]
            ).then_inc(out_sem, 16)
            sync.wait_ge(act_sem, NG)
            sync.dma_start(
                o_d[BPC - 1][:, HALF:], ot[:, BPC - 1, HALF:]
            ).then_inc(out_sem, 16)
            sync.wait_ge(out_sem, 16 * (BPC + 1))

        @block.vector
        def _(vector):
            # Zero the warm-up operand region: reading never-written SBUF
            # trips the sim (and is unhealthy on hardware).
            vector.memset(ot[:, 0, :128], 0).then_inc(warm_sem, 1)

        @block.tensor
        def _(tensor):
            # Warm the PE HAM clock gate (cold = 1.2 GHz) on zeros while
            # chunk 0 (weights + img0 rows 0-8) streams in.
            tensor.wait_ge(warm_sem, 1)
            for i in range(WARMUP):
                tensor.matmul(
                    ps[:, 7, :128], ot[:, 0, :128], ot[:, 0, :128],
                    start=True, stop=True,
                )
            prev_need = -1
            for j in range(NG):
                b, t = divmod(j, GPB)
                need = _chunk_needed(b, t)
                if need > prev_need:
                    tensor.wait_ge(c_sems[need], 16)
                    prev_need = need
                if t == 0 and 3 * b - 5 >= 1:
                    # banks for this image's 3 groups are free once ACT
                    # drained groups 8 back (one coarse wait per image)
                    tensor.wait_ge(act_sem, 3 * b - 5)
                X0, nX = XSPLIT[t]
                gr = allt[:, OFF_X + b * RCW : OFF_X + (b + 1) * RCW].rearrange(
                    "p (r c w) -> p r c w", r=WU, c=WU
                )
                for kk in range(9):
                    kx, ky = divmod(kk, 3)
                    rhs = gr[
                        :,
                        2 * X0 + kx : 2 * X0 + kx + 2 * nX - 1 : 2,
                        ky : ky + 2 * WOUT - 1 : 2,
                        :,
                    ]
                    mm = tensor.matmul(
                        ps[:, j % 8, : nX * WOUT * H],
                        wtr[:, kk, :],
                        rhs,
                        start=(kk == 0),
                        stop=(kk == 8),
                    )
                mm.then_inc(pe_sem, 1)

        @block.scalar
        def _(scalar):
            for j in range(NG):
                b, t = divmod(j, GPB)
                X0, nX = XSPLIT[t]
                off = X0 * WOUT * H
                scalar.wait_ge(pe_sem, j + 1)
                scalar.activation(
                    ot[:, b, off : off + nX * WOUT * H],
                    ps[:, j % 8, : nX * WOUT * H],
                    mybir.ActivationFunctionType.Copy,
                ).then_inc(act_sem, 1)
                if t == GPB - 1 and b < BPC - 1:
                    # image complete; ship it (the wait orders the async DMA
                    # read after this engine's writes for the race checker)
                    scalar.wait_ge(act_sem, j + 1)
                    scalar.dma_start(o_d[b], ot[:, b, :]).then_inc(out_sem, 16)

    return nc


def _prep_inputs(x, W, bias):
    """Build per-core [128, NELEM] fp16 input buffers.

    fp16: PE runs fp32 matmuls as LOW_HIGH double passes; fp16 is single-pass
    with fast-weight-load, and halves the dominant HBM traffic. Max rel err
    ~5e-4 at this contraction depth (fp32 PSUM accumulation).
    """
    x = np.asarray(x, dtype=np.float32)
    # xp[b, cin*4+v, (r*27+c)*4+w] = x[b,cin,r,c,4v+w], r/c < 27
    xp = np.ascontiguousarray(
        x.reshape(B, CIN, WIN, WIN, H, H)[:, :, :WU, :WU]
        .transpose(0, 1, 4, 2, 3, 5)
    ).reshape(B, CIN * H, RCW).astype(np.float16)
    # W: (1, 288, 32, 1, 1, 4, 4); p = cin*9 + kx*3 + ky
    # wt_sb[cin*4+v, kk*128 + o*4+u] = Wm[cin*9+kk, o, u, v]
    Wm = np.asarray(W, dtype=np.float32).reshape(CIN, KK * KK, COUT, H, H)
    wt_sb = np.ascontiguousarray(
        Wm.transpose(0, 4, 1, 2, 3)  # cin, v, kk, o, u
    ).reshape(128, 9 * 128).astype(np.float16)
    bufs = []
    for core in range(NCORES):
        shard = xp[core * BPC : (core + 1) * BPC]  # (BPC, 128, RCW)
        bufs.append(
            np.ascontiguousarray(
                np.concatenate(
                    [wt_sb, shard.transpose(1, 0, 2).reshape(128, BPC * RCW)],
                    axis=1,
                )
            )
        )
    return bufs


def _make_in_maps(x, W, bias):
    return [{"xin": buf} for buf in _prep_inputs(x, W, bias)]


def _unprep_output(full, bias):
    # full: (B, 128, NOUT) fp16 with partition o*4+u, free (X, Y, w).
    # Bias (a per-channel constant) is added host-side to keep the device
    # eviction a plain fp16 Copy.
    out = (
        full.astype(np.float32)
        .reshape(B, COUT, H, WOUT, WOUT, H)
        .transpose(0, 1, 3, 4, 2, 5)
        .reshape(B, COUT, WOUT, WOUT, HH)
    )
    out += np.asarray(bias, dtype=np.float32).reshape(1, COUT, 1, 1, 1)
    return np.ascontiguousarray(out)


def run_device(in_maps, trace=False, tmpdir=None):
    from concourse.bass_utils import run_bass_kernel_spmd

    if "nc" not in _cache:
        _cache["nc"] = _build_bass()
    return run_bass_kernel_spmd(
        _cache["nc"], in_maps, list(range(NCORES)), trace=trace, tmpdir=tmpdir
    )


def kernel(x, W, bias):
    in_maps = _make_in_maps(x, W, bias)
    res = run_device(in_maps, trace=False)
    full = np.concatenate(
        [np.asarray(res.results[i]["out"]) for i in range(NCORES)], axis=0
    )
    return _unprep_output(full, bias)


# revision 36
# speedup vs baseline: 1.0886x; 1.0068x over previous
"""Trainium2 Bass kernel for nn_Conv2dGeneral (capsule-style 4x4-pose conv).

Math (from the reference):
  out[b,o,X,Y,u,w] = sum_{cin,kx,ky,v} Wm[(cin,kx,ky),o,u,v] * x[b,cin,2X+kx,2Y+ky,4v+w] + bias[o]

Mapped to the PE array as a single 1152-deep contraction:
  K = (cin, v)  x  9 accumulation chunks over (kx, ky)   [9 x 128]
  M = (o, u)                                              [128 PSUM partitions]
  N = (X, Y, w)                                           [676 per batch image]

Data-parallel across 8 NeuronCores on the batch dim (8 images per core).

Pipelining: weights + all 8 images are packed into ONE fp16 DRAM buffer per
core, streamed in-order on the SP HWDGE ring via column-range DMA chunks
(weights + img0 rows first, img0/img1 split at row granularity so compute
starts while later images stream, imgs 2-7 whole; one completion semaphore
per chunk — cumulative counts race across queued DMAs). Row/col 27 of x are
dead (stride-2 3-tap windows over 28 touch only 0..26) and dropped
host-side. The PE warms its HAM clock gate on a memset scratch region,
sized to bridge exactly until chunk 0 lands (an idle gap before the first
real matmul would reset the HAM busy-window and halve the clock for the
first image). Outputs are evicted per-group in fp16 (bias re-added
host-side); images 0-6 ship from the ACT ring, image 7 ships as two halves
triggered by the otherwise-idle SP engine so the final transfer starts the
moment its eviction lands.
"""

import numpy as np

B, CIN, COUT = 64, 32, 32
KK, STRIDE = 3, 2
WIN, HH = 28, 16
H = 4
WU = 27                          # used rows/cols (row 27 never read)
WOUT = (WIN - KK) // STRIDE + 1  # 13
NCORES = 8
BPC = B // NCORES                # batches per core
RCW = WU * WU * H                # 2916 free elements per (cin,v) partition
RL = WU * H                      # 108 elems per row
NOUT = WOUT * WOUT * H           # 676 outputs per (o,u) partition per image
XSPLIT = ((0, 4), (4, 4), (8, 5))  # X groups per image (208/208/260 cols)
GPB = len(XSPLIT)
NG = GPB * BPC                   # 24 accumulation groups
HALF = 2 * WOUT * H * 4          # 416: output cols of the first two X groups
WARMUP = 42                      # PE warm-up matmuls; must BRIDGE until chunk 0
                                 # lands (an idle gap before the first real
                                 # matmul resets the HAM busy-window and the
                                 # whole first image then runs at 1.2 GHz)

OFF_X = 9 * 128                  # [wt(1152) | img0..7(2916 each)]
NELEM = OFF_X + BPC * RCW

# DMA chunks (elem ranges) on the SP ring: img0 in row-thirds matching
# XSPLIT row needs (X rows 0-3 read x-rows 0-8, X 4-7 read 8-16, X 8-12
# read 16-26), img1 in a 9/18-row split (kills the early-pipeline bubble:
# its first rows land before img0's compute drains), imgs 2-7 whole.
# The weights stream CONCURRENTLY on the ACT ring. Each chunk has its OWN
# completion semaphore: a single cumulative sem is racy because the 16
# per-SDMA-engine increments of back-to-back DMAs interleave, so sem>=16
# does not imply the FIRST dma finished. Per-engine FIFO order does make
# "chunk c done" imply all earlier chunks done on that ring.
_X1 = OFF_X + RCW
_CHUNKS = [
    (0, OFF_X + 9 * RL),                 # weights + img0 rows 0-8
    (OFF_X + 9 * RL, OFF_X + 17 * RL),
    (OFF_X + 17 * RL, OFF_X + RCW),
    (_X1, _X1 + 9 * RL),
    (_X1 + 9 * RL, _X1 + RCW),
]
for _b in range(2, BPC):
    _o = OFF_X + _b * RCW
    _CHUNKS.append((_o, _o + RCW))
NCHUNK = len(_CHUNKS)


def _chunk_needed(b, t):
    # index of the last DMA chunk group (b, t) requires
    if b == 0:
        return t
    if b == 1:
        return 3 if t == 0 else 4
    return b + 3

_cache = {}


def _build_bass():
    """Raw-bass build (no Tile): this toolchain's walrus codegen allows only
    ONE sync-wait per instruction, so all cross-engine sync is explicit
    single-sem waits; ordering beyond that rides on hardware transitivity.

    Engines: SP triggers 11 input DMA chunks then 8 per-image output DMAs
    (same HWDGE ring: FIFO keeps outputs behind all inputs), PE runs 25
    accumulation groups of 9 matmuls (one per kernel tap), ACT evicts
    PSUM->SBUF in fp16.
    """
    from contextlib import ExitStack

    import concourse.bass as bass
    import concourse.mybir as mybir

    f32 = mybir.dt.float32
    f16 = mybir.dt.float16

    nc = bass.Bass()
    xin = nc.declare_dram_parameter("xin", [128, NELEM], f16, isOutput=False)
    o_d = nc.declare_dram_parameter("out", [BPC, 128, NOUT], f16, isOutput=True)

    with (
        ExitStack() as stack,
        nc.sbuf_tensor([128, NELEM], f16) as allt,
        nc.sbuf_tensor([128, BPC, NOUT], f16) as ot,
        nc.psum_tensor([128, 8, 512], f32) as ps,
        nc.semaphore("pe_sem") as pe_sem,
        nc.semaphore("act_sem") as act_sem,
        nc.semaphore("out_sem") as out_sem,
        nc.semaphore("warm_sem") as warm_sem,
        nc.Block(no_gpsimd_drain=True) as block,
    ):
        c_sems = [
            stack.enter_context(nc.semaphore(f"c_sem{i}")) for i in range(NCHUNK)
        ]
        wtr = allt[:, 0 : 9 * 128].rearrange("p (k m) -> p k m", k=9)

        @block.sync
        def _(sync):
            for c, (a0, a1) in enumerate(_CHUNKS):
                sync.dma_start(allt[:, a0:a1], xin[:, a0:a1]).then_inc(c_sems[c], 16)
            # Ship the LAST image in two halves from here: SP is idle, so
            # the final (small) transfer starts the moment its eviction
            # lands instead of queueing behind ACT's trigger issue.
            sync.wait_ge(act_sem, NG - 1)
            sync.dma_start(
                o_d[BPC - 1][:, :HALF], ot[:, BPC - 1, :HALF]
            ).then_inc(out_sem, 16)
            sync.wait_ge(act_sem, NG)
            sync.dma_start(
                o_d[BPC - 1][:, HALF:], ot[:, BPC - 1, HALF:]
            ).then_inc(out_sem, 16)
            sync.wait_ge(out_sem, 16 * (BPC + 1))

        @block.vector
        def _(vector):
            # Zero the warm-up operand region: reading never-written SBUF
            # trips the sim (and is unhealthy on hardware).
            vector.memset(ot[:, 0, :128], 0).then_inc(warm_sem, 1)

        @block.tensor
        def _(tensor):
            # Warm the PE HAM clock gate (cold = 1.2 GHz) on zeros while
            # chunk 0 (weights + img0 rows 0-8) streams in.
            tensor.wait_ge(warm_sem, 1)
            for i in range(WARMUP):
                tensor.matmul(
                    ps[:, 7, :128], ot[:, 0, :128], ot[:, 0, :128],
                    start=True, stop=True,
                )
            prev_need = -1
            for j in range(NG):
                b, t = divmod(j, GPB)
                need = _chunk_needed(b, t)
                if need > prev_need:
                    tensor.wait_ge(c_sems[need], 16)
                    prev_need = need
                if t == 0 and 3 * b - 5 >= 1:
                    # banks for this image's 3 groups are free once ACT
                    # drained groups 8 back (one coarse wait per image)
                    tensor.wait_ge(act_sem, 3 * b - 5)
                X0, nX = XSPLIT[t]
                gr = allt[:, OFF_X + b * RCW : OFF_X + (b + 1) * RCW].rearrange(
                    "p (r c w) -> p r c w", r=WU, c=WU
                )
                for kk in range(9):
                    kx, ky = divmod(kk, 3)
                    rhs = gr[
                        :,
                        2 * X0 + kx : 2 * X0 + kx + 2 * nX - 1 : 2,
                        ky : ky + 2 * WOUT - 1 : 2,
                        :,
                    ]
                    mm = tensor.matmul(
                        ps[:, j % 8, : nX * WOUT * H],
                        wtr[:, kk, :],
                        rhs,
                        start=(kk == 0),
                        stop=(kk == 8),
                    )
                mm.then_inc(pe_sem, 1)

        @block.scalar
        def _(scalar):
            for j in range(NG):
                b, t = divmod(j, GPB)
                X0, nX = XSPLIT[t]
                off = X0 * WOUT * H
                scalar.wait_ge(pe_sem, j + 1)
                scalar.activation(
                    ot[:, b, off : off + nX * WOUT * H],
                    ps[:, j % 8, : nX * WOUT * H],
                    mybir.ActivationFunctionType.Copy,
                ).then_inc(act_sem, 1)
                if t == GPB - 1 and b < BPC - 1:
                    # image complete; ship it (the wait orders the async DMA
                    # read after this engine's writes for the race checker)
                    scalar.wait_ge(act_sem, j + 1)
                    scalar.dma_start(o_d[b], ot[:, b, :]).then_inc(out_sem, 16)

    return nc


def _prep_inputs(x, W, bias):
    """Build per-core [128, NELEM] fp16 input buffers.

    fp16: PE runs fp32 matmuls as LOW_HIGH double passes; fp16 is single-pass
    with fast-weight-load, and halves the dominant HBM traffic. Max rel err
    ~5e-4 at this contraction depth (fp32 PSUM accumulation).
    """
    x = np.asarray(x, dtype=np.float32)
    # xp[b, cin*4+v, (r*27+c)*4+w] = x[b,cin,r,c,4v+w], r/c < 27
    xp = np.ascontiguousarray(
        x.reshape(B, CIN, WIN, WIN, H, H)[:, :, :WU, :WU]
        .transpose(0, 1, 4, 2, 3, 5)
    ).reshape(B, CIN * H, RCW).astype(np.float16)
    # W: (1, 288, 32, 1, 1, 4, 4); p = cin*9 + kx*3 + ky
    # wt_sb[cin*4+v, kk*128 + o*4+u] = Wm[cin*9+kk, o, u, v]
    Wm = np.asarray(W, dtype=np.float32).reshape(CIN, KK * KK, COUT, H, H)
    wt_sb = np.ascontiguousarray(
        Wm.transpose(0, 4, 1, 2, 3)  # cin, v, kk, o, u
    ).reshape(128, 9 * 128).astype(np.float16)
    bufs = []
    for core in range(NCORES):
        shard = xp[core * BPC : (core + 1) * BPC]  # (BPC, 128, RCW)
        bufs.append(
            np.ascontiguousarray(
                np.concatenate(
                    [wt_sb, shard.transpose(1, 0, 2).reshape(128, BPC * RCW)],
                    axis=1,
                )
            )
        )
    return bufs


def _make_in_maps(x, W, bias):
    return [{"xin": buf} for buf in _prep_inputs(x, W, bias)]


def _unprep_output(full, bias):
    # full: (B, 128, NOUT) fp16 with partition o*4+u, free (X, Y, w).
    # Bias (a per-channel constant) is added host-side to keep the device
    # eviction a plain fp16 Copy.
    out = (
        full.astype(np.float32)
        .reshape(B, COUT, H, WOUT, WOUT, H)
        .transpose(0, 1, 3, 4, 2, 5)
        .reshape(B, COUT, WOUT, WOUT, HH)
    )
    out += np.asarray(bias, dtype=np.float32).reshape(1, COUT, 1, 1, 1)
    return np.ascontiguousarray(out)


def run_device(in_maps, trace=False, tmpdir=None):
    from concourse.bass_utils import run_bass_kernel_spmd

    if "nc" not in _cache:
        _cache["nc"] = _build_bass()
    return run_bass_kernel_spmd(
        _cache["nc"], in_maps, list(range(NCORES)), trace=trace, tmpdir=tmpdir
    )


def kernel(x, W, bias):
    in_maps = _make_in_maps(x, W, bias)
    res = run_device(in_maps, trace=False)
    full = np.concatenate(
        [np.asarray(res.results[i]["out"]) for i in range(NCORES)], axis=0
    )
    return _unprep_output(full, bias)
